# revision 25
# baseline (speedup 1.0000x reference)
"""Trainium2 Bass kernel for nn_DDSTSTransformer_9663676416718.

Data-parallel over batch B=32 across 8 cores (4 batches/core), params
replicated. Per-core activations live as [128, 512] tiles = 4 batches
stacked along partitions (32 channels each).

Structure (v2 — ACT-roofline design):
  - The scalar engine's exp over the full [512,512] score matrix per
    (batch, head) is the hard roof (~18.4 us per batch-layer). The whole
    schedule is built to keep ACT 100% fed; the PE has ~2x slack.
  - DDS convs are host-fused into 15-tap full convs and run as im2col
    matmuls over a DMA-built 4-tap-packed tile (as v1).
  - Scores S = K^T K run row-tiled (4 heads concurrent via
    tile_position=(32h,0)).
  - AV runs col-tiled (4 heads concurrent via tile_position=(0,32h),
    M=32) into one PSUM bank, giving attn in [4h*32c, 512] layout that
    feeds unify directly. A parallel col-tiled ones-matmul accumulates
    the softmax denominator Z into a second bank, 32x-replicated per
    head, so the divide is a single reciprocal_approx_fast + tensor_mul
    on [128, 512] with zero DMA plumbing.
  - Per-batch-band instance norms + ff let each batch's layer tail and
    next-layer conv start while other batches still run attention, so
    ACT never drains at layer boundaries.
  - Emission is software-pipelined: scores(u+1) before AV(u), tails and
    next-layer convs interleaved into the attention unit stream.
"""

import numpy as np

B, CIN, L = 32, 6, 512
TC, H, D, DM = 32, 8, 6, 4
EPS = 1e-5
NCORES = 8
BPC = B // NCORES  # batches per core
KT = 15            # fused dds conv taps
PAD = (KT - 1) // 2

# Conv weights (bf16 wconv blob, [128, 2048]): stored for im2col
# matmuls: partition = (tt, c) with tap t = 4*k + tt (padded to 16 taps,
# tap 15 zero), k in the free dim.
QKOFF = 0                  # [4k, 256] fused qk conv im2col lhsT
VOFF = QKOFF + 4 * 256     # [4k, 256] fused v conv im2col rhs
NWC = VOFF + 4 * 256
# fp32 wrest blob [128, NWR]
FF1OFF = 0                 # [128] ff1 lhsT, band-replicated
FF2OFF = FF1OFF + 128      # [32]  ff2 lhsT (full 128 partitions)
UOFF = FF2OFF + 32         # [2, 32] unify lhsT per ktile
GBOFF = UOFF + 64          # 4 cols: n1_g, n1_b, n2_g, n2_b (tiled x4)
NWR = GBOFF + 4
XPW = L + 16               # padded input width (7 left, 9 right zeros)

_CACHE = {}


def _split_excess_waits(nc, mybir, limits):
    """Walrus's TPB encodings accept a limited number of sync-wait
    commands per instruction (1 for Matmult/LDWEIGHTS on this build).
    Move excess waits onto freshly inserted same-engine NoOps directly
    before the instruction — identical engine-stream semantics, valid
    encoding."""
    for fn in nc.m.functions:
        for blk in fn.blocks:
            il = blk.instructions
            i = 0
            while i < len(il):
                inst = il[i]
                keep = limits.get(type(inst).__name__, 1)
                si = getattr(inst, 'sync_info', None)
                if si is not None and si.on_wait and len(si.on_wait) > keep:
                    waits = list(si.on_wait)
                    extra, rest = waits[:-keep], waits[-keep:]
                    nops = []
                    for w in extra:
                        n = mybir.InstNoOp(name=f'I-wsplit-{nc.next_id()}',
                                           ins=[], outs=[])
                        n.engine = inst.engine
                        n.sync_info = mybir.SyncInfo(on_wait=[w], on_update=[])
                        nops.append(n)
                    inst.sync_info = mybir.SyncInfo(
                        on_wait=rest, on_update=list(si.on_update))
                    il[i:i] = nops
                    i += len(nops)
                i += 1


def _fuse_dds(pw, dwa, dwb, gate):
    """Fold pointwise + gated depthwise pair into W[t, c, o]."""
    g = np.exp(gate - gate.max())
    g = g / g.sum()
    O = pw.shape[0]
    ka, kb = dwa.shape[1], dwb.shape[1]
    k = max(ka, kb)
    wc = np.zeros((O, k), np.float32)
    wc[:, (k - ka) // 2:(k - ka) // 2 + ka] += g[0] * dwa
    wc[:, (k - kb) // 2:(k - kb) // 2 + kb] += g[1] * dwb
    # W[t, c, o] = pw[o, c] * wc[o, t]
    return np.einsum('oc,ot->tco', pw, wc).astype(np.float32)


def _rep4(a):
    """Tile a [32, ...] band 4x along partitions -> [128, ...]."""
    return np.concatenate([a] * 4, axis=0)


def _host_prep(inputs):
    """Build per-core input maps (numpy only)."""
    import ml_dtypes
    BF = ml_dtypes.bfloat16
    F8 = ml_dtypes.float8_e4m3
    x = np.asarray(inputs['x'], np.float32)

    wconv = np.zeros((D, 128, NWC), F8)
    wrest = np.zeros((D, 128, NWR), np.float32)
    for l in range(D):
        wqk = _fuse_dds(np.asarray(inputs['qk_pw'][l]), np.asarray(inputs['qk_dwa'][l]),
                        np.asarray(inputs['qk_dwb'][l]), np.asarray(inputs['qk_gate'][l]))
        wv = _fuse_dds(np.asarray(inputs['v_pw'][l]), np.asarray(inputs['v_dwa'][l]),
                       np.asarray(inputs['v_dwb'][l]), np.asarray(inputs['v_gate'][l]))
        # [t, c, o] -> partition (tt, c) = 32*tt + c, free (k, o) = 256*k + o
        for W, off in ((wqk, QKOFF), (wv, VOFF)):
            W16 = np.zeros((16, TC, 256), np.float32)
            W16[:KT] = W
            if l == 0:
                # compose the pointwise encoder into the layer-0 convs so
                # they read the raw input directly (encoder stays only on
                # the residual path, off the startup critical chain)
                Wc = np.einsum('tco,cd->tdo', W16,
                               np.asarray(inputs['enc_w'], np.float32))
                W16 = np.zeros((16, TC, 256), np.float32)
                W16[:, :CIN] = Wc
            # x8: keeps small fused weights out of the e4m3 denormal
            # range; the 64x score scale folds into the exp scale and the
            # 8x v scale into the vt drain
            wconv[l, :, off:off + 1024] = (
                (8.0 * W16).reshape(4, 4, TC, 256)  # [k, tt, c, o]
                   .transpose(1, 2, 0, 3)           # [tt, c, k, o]
                   .reshape(128, 1024).astype(F8))
        wrest[l, :, FF1OFF:FF1OFF + 128] = _rep4(np.asarray(inputs['ff_w1'][l]).T)   # [c, m]
        wrest[l, :, FF2OFF:FF2OFF + 32] = np.asarray(inputs['ff_w2'][l]).T           # [dm, o]
        # unify lhsT: u[hc_lo, kt*32+o] = unify_w[o, kt*128+hc_lo]
        ut = np.asarray(inputs['unify_w'][l]).T.reshape(2, 128, TC).transpose(1, 0, 2)
        wrest[l, :, UOFF:UOFF + 64] = ut.reshape(128, 64)
        wrest[l, :, GBOFF + 0] = _rep4(np.asarray(inputs['n1_g'][l]))
        wrest[l, :, GBOFF + 1] = _rep4(np.asarray(inputs['n1_b'][l]))
        wrest[l, :, GBOFF + 2] = _rep4(np.asarray(inputs['n2_g'][l]))
        wrest[l, :, GBOFF + 3] = _rep4(np.asarray(inputs['n2_b'][l]))

    encw = np.zeros((128, TC), np.float32)
    for i in range(4):
        encw[32 * i:32 * i + CIN, :] = np.asarray(inputs['enc_w']).T  # [c, o]

    wlog = _fuse_dds(np.asarray(inputs['log_pw']), np.asarray(inputs['log_dwa']),
                     np.asarray(inputs['log_dwb']), np.asarray(inputs['log_gate']))  # [3, 32, 1]
    wtail = _rep4(wlog[:, :, 0].T)  # [c, t] -> [128, 3]

    in_maps = []
    for core in range(NCORES):
        xin4 = np.zeros((128, L), np.float32)
        for i in range(BPC):
            xin4[32 * i:32 * i + CIN, :] = x[BPC * core + i]
        in_maps.append({
            'xin4': xin4,
            'encw': encw,
            'wconv': wconv,
            'wrest': wrest,
            'wtail': wtail,
        })
    return in_maps


def build_nc(debug=False, split_waits=True):
    import concourse.bass as bass
    import concourse.mybir as mybir
    import concourse.tile as tile

    FP32 = mybir.dt.float32
    BF16 = mybir.dt.bfloat16
    AF = mybir.ActivationFunctionType
    ALU = mybir.AluOpType
    INV_SQRT_C = float(TC) ** -0.5
    F32R = mybir.dt.float32r
    I16 = mybir.dt.int16
    FP8 = mybir.dt.float8e4
    WSC2 = 1.0 / 64.0          # qk conv weights are stored x8
    SCH_A = (128.0 / np.log(2.0)) * INV_SQRT_C * WSC2
    SCH_B = 127.0 * 128.0 - 8.5

    nc = bass.Bass()
    xin4_d = nc.declare_dram_parameter('xin4', [128, L], FP32, isOutput=False)
    encw_d = nc.declare_dram_parameter('encw', [128, TC], FP32, isOutput=False)
    wconv_d = nc.declare_dram_parameter('wconv', [D, 128, NWC], FP8, isOutput=False)
    wrest_d = nc.declare_dram_parameter('wrest', [D, 128, NWR], FP32, isOutput=False)
    wtail_d = nc.declare_dram_parameter('wtail', [128, 3], FP32, isOutput=False)
    out_d = nc.declare_dram_parameter('out', [BPC, L], FP32, isOutput=True)
    dbg = {}
    if debug:
        for name, shape in [('dbg_enc', [128, L]), ('dbg_qk', [128, L]),
                            ('dbg_vt', [128, 256]), ('dbg_exps', [128, L]),
                            ('dbg_z', [128, L]),
                            ('dbg_attn', [128, L]), ('dbg_l0', [128, L])]:
            dbg[name] = nc.declare_dram_parameter(name, shape, FP32, isOutput=True)

    with tile.TileContext(nc) as tc:
        with (
            tc.tile_pool(name='pconst', bufs=1) as pconst,
            tc.tile_pool(name='pw', bufs=3) as pw,
            tc.tile_pool(name='pqk', bufs=12) as pqk,
            tc.tile_pool(name='pvt', bufs=20) as pvt,
            tc.tile_pool(name='pek', bufs=8) as pek,
            tc.tile_pool(name='pat', bufs=5) as pat,
            tc.tile_pool(name='prz', bufs=2) as prz,
            tc.tile_pool(name='pa', bufs=4) as pa,
            tc.tile_pool(name='ppad', bufs=2) as ppad,
            tc.tile_pool(name='pxc', bufs=3) as pxc,
            tc.tile_pool(name='psm', bufs=4) as psm,
            tc.tile_pool(name='pps', bufs=2, space='PSUM') as pps,
            tc.tile_pool(name='pz', bufs=2, space='PSUM') as pz,
            tc.tile_pool(name='pcv', bufs=2, space='PSUM') as pcv,
        ):

            def mm(out, lhsT, rhs, **kw):
                # float32r streams 1 col/cycle at N>=256 (fp32 is 4x slower)
                nc.tensor.matmul(out=out, lhsT=lhsT.bitcast(F32R),
                                 rhs=rhs.bitcast(F32R), **kw)

            eps_t = pconst.tile([128, 1], FP32, tag='eps')
            nc.vector.memset(eps_t, EPS)
            zeros16 = pconst.tile([128, 16], FP32, tag='zeros16')
            nc.vector.memset(zeros16, 0.0)
            ones32 = pconst.tile([128, 32], BF16, tag='ones32')
            nc.vector.memset(ones32, 1.0)
            xin4 = pconst.tile([128, L], FP32, tag='xin4')
            encw = pconst.tile([128, TC], FP32, tag='encw')
            wtail = pconst.tile([128, 3], FP32, tag='wtail')
            nc.sync.dma_start(out=xin4.bitcast(F32R), in_=xin4_d[:, :].bitcast(F32R))

            # preload the natural_log_exp ACT table set during startup DMAs
            warm = psm.tile([128, 1], FP32, tag='warm', bufs=1)
            nc.scalar.activation(out=warm, in_=eps_t, func=AF.Exp, bias=0.0,
                                 scale=1.0)

            def load_weights(l):
                wc = pw.tile([128, NWC], FP8, tag='wcsb')
                for a, b in ((QKOFF, VOFF), (VOFF, NWC)):
                    nc.sync.dma_start(out=wc[:, a:b], in_=wconv_d[l, :, a:b])
                wr = pw.tile([128, NWR], FP32, tag='wrsb')
                nc.sync.dma_start(out=wr.bitcast(F32R),
                                  in_=wrest_d[l, :, :].bitcast(F32R))
                return (wc, wr)

            wsb = {0: load_weights(0)}
            # encw/wtail after layer-0 weights: nothing needs them early
            nc.sync.dma_start(out=encw.bitcast(F32R), in_=encw_d[:, :].bitcast(F32R))
            nc.sync.dma_start(out=wtail.bitcast(F32R), in_=wtail_d[:, :].bitcast(F32R))

            # ---------- encoder ----------
            enct = pcv.tile([128, L], FP32, tag='cv', name='enct')
            encp = enct
            for i in range(4):
                nc.tensor.matmul(
                    out=encp[32 * i:32 * i + 32, :],
                    lhsT=encw[32 * i:32 * i + 32, :],
                    rhs=xin4[32 * i:32 * i + 32, :],
                    start=True, stop=True,
                    tile_position=(32 * i, 32 * i))
            enc4 = pa.tile([128, L], FP32, tag='out4', bufs=4, name='enc4')
            nc.vector.tensor_copy(out=enc4, in_=encp)
            if debug:
                nc.sync.dma_start(out=dbg['dbg_enc'][:, :], in_=enc4)

            # pipeline state
            OUT4 = {0: enc4}   # OUT4[l] = input residual stream for layer l
            MID = {}           # MID[l] = post-norm1 stream of layer l
            Q, VT, XP, AT = {}, {}, {}, {}

            def build_xpad_band(l, bi):
                if l not in XP:
                    XP[l] = ppad.tile([128, XPW], FP32, tag='xpad',
                                      name=f'xpad{l}')
                xp = XP[l]
                src = xin4 if l == 0 else OUT4[l]
                p0, p1 = 32 * bi, 32 * bi + 32
                nc.vector.tensor_copy(out=xp[p0:p1, 0:PAD].bitcast(F32R),
                                      in_=zeros16[0:32, 0:PAD])
                nc.vector.tensor_copy(out=xp[p0:p1, PAD + L:].bitcast(F32R),
                                      in_=zeros16[0:32, 0:XPW - PAD - L])
                nc.vector.tensor_copy(out=xp[p0:p1, PAD:PAD + L].bitcast(F32R),
                                      in_=src[p0:p1, :])
                return xp

            XC = {}

            def conv_dma(l, bi):
                """im2col gather for (layer l, batch bi) — prefetch stage.
                gpsimd (SWDGE) so the gather casts fp32 -> bf16 in flight."""
                xp = build_xpad_band(l, bi)
                xc = pxc.tile([128, 4, L], FP8, tag='xc', name=f'xc{l}_{bi}')
                for tt in range(4):
                    sl = xp[32 * bi:32 * bi + 32, tt:tt + L]
                    src_ap = bass.AP(
                        tensor=sl.tensor, offset=sl.offset,
                        ap=[sl.ap[0], [4, 4], [1, L]])
                    nc.gpsimd.dma_start(out=xc[32 * tt:32 * tt + 32, :, :],
                                        in_=src_ap)
                XC[(l, bi)] = xc

            def conv_qk(l, bi, j):
                """qk conv half j for (l, bi): 4 matmuls + bf16 drain."""
                w, _ = wsb[l]
                xc = XC[(l, bi)]
                qp = pcv.tile([128, L], FP32, tag='cv', name=f'qp{l}_{bi}_{j}')
                for p in range(2):
                    wsl = w[:, QKOFF + 512 * p + 128 * j:
                            QKOFF + 512 * p + 128 * j + 128]
                    wdr = bass.AP(tensor=wsl.tensor, offset=wsl.offset,
                                  ap=[wsl.ap[0], [256, 2], [1, 128]])
                    nc.tensor.matmul(
                       out=qp, lhsT=wdr, rhs=xc[:, 2 * p:2 * p + 2, :],
                       start=(p == 0), stop=(p == 1),
                       perf_mode=mybir.MatmulPerfMode.DoubleRow)
                q = pqk.tile([128, L], BF16, tag='qk', name=f'q{l}_{bi}_{j}')
                nc.vector.tensor_copy(out=q, in_=qp)
                Q.setdefault((l, bi), []).append(q)

            def conv_v(l, bi, half):
                """v conv half (2 l-tiles) for (l, bi), transposed out."""
                w, _ = wsb[l]
                xc = XC[(l, bi)]
                vp = pcv.tile([128, 2, 256], FP32, tag='cv',
                              name=f'vp{l}_{bi}_{half}')
                for i in range(2):
                    lt = 2 * half + i
                    for p in range(2):
                        wsl = w[:, VOFF + 512 * p:VOFF + 512 * p + 256]
                        wdr = bass.AP(tensor=wsl.tensor, offset=wsl.offset,
                                      ap=[wsl.ap[0], [256, 2], [1, 256]])
                        nc.tensor.matmul(
                           out=vp[:, i, :],
                           lhsT=xc[:, 2 * p:2 * p + 2, 128 * lt:128 * lt + 128],
                           rhs=wdr,
                           start=(p == 0), stop=(p == 1),
                           perf_mode=mybir.MatmulPerfMode.DoubleRow)
                for i in range(2):
                    # bf16: walrus rejects fp32r col-tiled matmuls, and the
                    # AV/Z stage col-tiles; the x8 weight scale comes out in
                    # this PSUM drain
                    v = pvt.tile([128, 256], BF16, tag='vt',
                                 name=f'v{l}_{bi}_{2 * half + i}')
                    nc.vector.tensor_scalar(out=v, in0=vp[:, i, :],
                                            scalar1=0.125, scalar2=None,
                                            op0=ALU.mult)
                    VT.setdefault((l, bi), []).append(v)
                if half == 1:
                    XC.pop((l, bi))

            def scores_k(l, bi, g, k, eks):
                """Score matmuls + exp for head group g, key-tile k."""
                qk = Q[(l, bi)][g]
                psA = pps.tile([128, 2, L], FP32, tag='s',
                               name=f'sA{l}_{bi}_{g}_{k}')
                psB = pps.tile([128, 2, L], FP32, tag='s',
                               name=f'sB{l}_{bi}_{g}_{k}')
                for hh in range(4):
                    ps, half = (psA, hh) if hh < 2 else (psB, hh - 2)
                    nc.tensor.matmul(
                       out=ps[:, half, :],
                       lhsT=qk[32 * hh:32 * hh + 32, 128 * k:128 * k + 128],
                       rhs=qk[32 * hh:32 * hh + 32, :],
                       start=True, stop=True,
                       tile_position=(32 * hh, 0))
                ekk = pek.tile([128, 4, L], BF16, tag='ek',
                               name=f'ek{l}_{bi}_{g}_{k}')
                nc.scalar.activation(out=ekk[:, 0:2, :],
                                     in_=psA[:, :, :], func=AF.Exp,
                                     bias=0.0, scale=INV_SQRT_C * WSC2)
                if k == 3 or (k == 1 and g == 0):
                    # Schraudolph exp on DVE, directly in bf16 bit space:
                    # bf16_bits(e^x) ~ round(128/ln2 * x + (127*128 - 8.5)).
                    # The softmax ratio cancels the ~1.5% element error.
                    nc.vector.tensor_scalar(
                        out=ekk[:, 2:4, :].bitcast(I16),
                        in0=psB[:, :, :],
                        scalar1=SCH_A, scalar2=SCH_B,
                        op0=ALU.mult, op1=ALU.add)
                else:
                    nc.scalar.activation(out=ekk[:, 2:4, :],
                                         in_=psB[:, :, :], func=AF.Exp,
                                         bias=0.0, scale=INV_SQRT_C * WSC2)
                eks.append(ekk)

            def attend_k(st, k):
                """Col-tiled AV + replicated Z for key-tile k of unit st."""
                l, bi, g, eks = st['u'] + (st['eks'],)
                vt = VT[(l, bi)]
                if k == 0:
                    st['av'] = pz.tile([128, L], FP32, tag='z',
                                       name=f'av{l}_{bi}_{g}')
                    st['zz'] = pz.tile([128, L], FP32, tag='z',
                                       name=f'zz{l}_{bi}_{g}')
                av, zz = st['av'], st['zz']
                for h in range(4):
                    nc.tensor.matmul(
                        out=av[32 * h:32 * h + 32, :],
                        lhsT=vt[k][:, 32 * (4 * g + h):32 * (4 * g + h) + 32],
                        rhs=eks[k][:, h, :],
                        start=(k == 0), stop=(k == 3),
                        tile_position=(0, 32 * h))
                for h in range(4):
                    nc.tensor.matmul(
                        out=zz[32 * h:32 * h + 32, :],
                        lhsT=ones32,
                        rhs=eks[k][:, h, :],
                        start=(k == 0), stop=(k == 3),
                        tile_position=(0, 32 * h))

            def attend_fin(st):
                """Softmax divide: at = av * (1/zz)."""
                l, bi, g = st['u']
                rzb = prz.tile([128, L], FP32, tag='rz')
                nc.vector.reciprocal_approx_fast(out=rzb, in_=st['zz'])
                at = pat.tile([128, L], FP32, tag='at', name=f'at{l}_{bi}_{g}')
                nc.vector.tensor_mul(out=at.bitcast(F32R), in0=st['av'],
                                     in1=rzb)
                AT[(bi, g)] = at

            def norm_band(resb, dst, bi, w, goff, boff):
                """InstanceNorm on one 32-partition band: dst_band =
                resb*se + (beta - mu*se), se = gamma/sqrt(var+eps)."""
                p0, p1 = 32 * bi, 32 * bi + 32
                stats = psm.tile([128, 6], FP32, tag='stats', bufs=4)
                mv = psm.tile([128, 2], FP32, tag='mv', bufs=4)
                nc.vector.bn_stats(out=stats[p0:p1, :], in_=resb[p0:p1, :])
                nc.vector.bn_aggr(out=mv[p0:p1, :], in_=stats[p0:p1, :])
                lnv = psm.tile([128, 1], FP32, tag='lnv', bufs=4)
                rstd = psm.tile([128, 1], FP32, tag='rstd', bufs=4)
                nc.scalar.activation(out=lnv[p0:p1, :], in_=mv[p0:p1, 1:2],
                                     func=AF.Ln, bias=eps_t[p0:p1, :], scale=1.0)
                nc.scalar.activation(out=rstd[p0:p1, :], in_=lnv[p0:p1, :],
                                     func=AF.Exp, bias=0.0, scale=-0.5)
                se = psm.tile([128, 1], FP32, tag='se', bufs=4)
                tmp = psm.tile([128, 1], FP32, tag='tmp', bufs=4)
                bv = psm.tile([128, 1], FP32, tag='bv', bufs=4)
                nc.vector.tensor_mul(out=se[p0:p1, :], in0=rstd[p0:p1, :],
                                     in1=w[p0:p1, goff:goff + 1])
                nc.vector.tensor_mul(out=tmp[p0:p1, :], in0=mv[p0:p1, 0:1],
                                     in1=se[p0:p1, :])
                nc.vector.tensor_sub(out=bv[p0:p1, :], in0=w[p0:p1, boff:boff + 1],
                                     in1=tmp[p0:p1, :])
                nc.vector.tensor_scalar(out=dst[p0:p1, :].bitcast(F32R),
                                        in0=resb[p0:p1, :],
                                        scalar1=se[p0:p1, :],
                                        scalar2=bv[p0:p1, :],
                                        op0=ALU.mult, op1=ALU.add)

            def tail_a(l, bi):
                """unify + residual + norm1 for one batch band."""
                _, w = wsb[l]
                ub = pcv.tile([128, L], FP32, tag='cv', name=f'ub{l}_{bi}')
                for kt in range(2):
                    mm(out=ub[0:32, :],
                       lhsT=w[:, UOFF + kt * 32:UOFF + (kt + 1) * 32],
                       rhs=AT[(bi, kt)],
                       start=(kt == 0), stop=(kt == 1))
                if l not in MID:
                    MID[l] = pa.tile([128, L], FP32, tag='out4', bufs=4,
                                     name=f'mid{l}')
                resb = pa.tile([128, L], FP32, tag='res', bufs=3,
                               name=f'r1_{l}_{bi}')
                p0, p1 = 32 * bi, 32 * bi + 32
                nc.vector.tensor_add(out=resb[p0:p1, :], in0=ub[0:32, :],
                                     in1=OUT4[l][p0:p1, :])
                norm_band(resb, MID[l], bi, w, GBOFF + 0, GBOFF + 1)

            FFS = {}

            def tail_b1(l, bi):
                """ff expand + relu for one batch band."""
                _, w = wsb[l]
                p0, p1 = 32 * bi, 32 * bi + 32
                ffp = pcv.tile([128, L], FP32, tag='cv', name=f'ffp{l}_{bi}')
                mm(out=ffp,
                   lhsT=w[p0:p1, FF1OFF:FF1OFF + 128],
                   rhs=MID[l][p0:p1, :],
                   start=True, stop=True,
                   tile_position=(32 * bi, 0))
                ffs = pa.tile([128, L], FP32, tag='ffs', bufs=2,
                              name=f'ffs{l}_{bi}')
                nc.vector.tensor_scalar_max(out=ffs.bitcast(F32R), in0=ffp,
                                            scalar1=0.0)
                FFS[(l, bi)] = ffs


            def tail_b2(l, bi):
                """ff contract + residual + norm2 for one batch band."""
                _, w = wsb[l]
                p0, p1 = 32 * bi, 32 * bi + 32
                f2 = pcv.tile([128, L], FP32, tag='cv', name=f'f2_{l}_{bi}')
                mm(out=f2[0:32, :],
                   lhsT=w[:, FF2OFF:FF2OFF + 32],
                   rhs=FFS.pop((l, bi)),
                   start=True, stop=True)
                if (l + 1) not in OUT4:
                    OUT4[l + 1] = pa.tile([128, L], FP32, tag='out4', bufs=4,
                                          name=f'out{l + 1}')
                resb = pa.tile([128, L], FP32, tag='res', bufs=3,
                               name=f'r2_{l}_{bi}')
                nc.vector.tensor_add(out=resb[p0:p1, :], in0=f2[0:32, :],
                                     in1=MID[l][p0:p1, :])
                norm_band(resb, OUT4[l + 1], bi, w, GBOFF + 2, GBOFF + 3)

            l4c = pa.tile([1, BPC * L], FP32, tag='l4', bufs=1)

            def logits(bi):
                """to_logits DDS conv (3 taps) for one batch."""
                xp = build_xpad_band(D, bi)
                pst = pcv.tile([128, L], FP32, tag='cv', name=f'tail{bi}')
                for t in range(3):
                    mm(out=pst[0:1, :],
                       lhsT=wtail[32 * bi:32 * bi + 32, t:t + 1],
                       rhs=xp[32 * bi:32 * bi + 32, PAD - 1 + t:PAD - 1 + t + L],
                       start=(t == 0), stop=(t == 2),
                       tile_position=(32 * bi, 0))
                nc.vector.tensor_copy(out=l4c[0:1, bi * L:(bi + 1) * L],
                                      in_=pst[0:1, :])

            # ---------- pipelined emission (global flat schedule) ----------
            # per-k interleave: the PE stream alternates [scores_k(u) |
            # AV_k(u-1) | one background thunk], so the depth-2 score ring
            # never parks the exp stream and background work (tails, convs)
            # fills the PE while ACT drains each score tile. Convs are
            # split into 4 thunks, tails into 3; im2col DMAs prefetch one
            # slot ahead of the matmul thunks.
            from collections import deque
            THUNKS = deque()

            sched = {}
            for l in range(D):
                for b in range(BPC):
                    o = 8 * l + 2 * b
                    if l < D - 1:
                        sched.setdefault(o + 3, []).append(('ta', l, b))
                        sched.setdefault(o + 4, []).append(('tb', l, b))
                        sched.setdefault(o + 5, []).append(('nsd', l + 1, b))
                        sched.setdefault(o + 6, []).append(('nsm', l + 1, b))
                    else:
                        # last layer: pull tails/logits as early as the
                        # data deps allow, so only batch 3's chain trails
                        # the final exp
                        sched.setdefault(o + 2, []).append(('ta', l, b))
                        sched.setdefault(o + 3, []).append(('tb', l, b))
                        sched.setdefault(o + 3, []).append(('nsd', D, b))
                        sched.setdefault(o + 3, []).append(('nsm', D, b))

            def enqueue_sched(j):
                for kind, a0, a1 in sched.pop(j, []):
                    if kind == 'ta':
                        THUNKS.append(lambda a0=a0, a1=a1: tail_a(a0, a1))
                    elif kind == 'tb':
                        THUNKS.append(lambda a0=a0, a1=a1: tail_b1(a0, a1))
                        THUNKS.append(lambda a0=a0, a1=a1: tail_b2(a0, a1))
                    elif kind == 'nsd':
                        if a0 < D:
                            THUNKS.append(
                                lambda a0=a0, a1=a1: conv_dma(a0, a1))
                        else:
                            THUNKS.append(
                                lambda a1=a1: build_xpad_band(D, a1))
                    elif kind == 'cqk':
                        for jj in (0, 1):
                            THUNKS.append(
                                lambda a0=a0, a1=a1, jj=jj:
                                    conv_qk(a0, a1, jj))
                    elif kind == 'cv':
                        for jj in (0, 1):
                            THUNKS.append(
                                lambda a0=a0, a1=a1, jj=jj:
                                    conv_v(a0, a1, jj))
                    else:
                        if a0 < D:
                            for jj, fn in ((0, conv_qk), (1, conv_qk),
                                           (0, conv_v), (1, conv_v)):
                                THUNKS.append(
                                    lambda a0=a0, a1=a1, jj=jj, fn=fn:
                                        fn(a0, a1, jj))
                        else:
                            THUNKS.append(lambda a1=a1: logits(a1))

            def conv_all(l, bi):
                conv_qk(l, bi, 0)
                conv_qk(l, bi, 1)
                conv_v(l, bi, 0)
                conv_v(l, bi, 1)

            # layer-0 convs read xin4 directly (encoder composed on host)
            conv_dma(0, 0)
            conv_all(0, 0)
            conv_dma(0, 1)
            conv_all(0, 1)
            units = [(l, bi, g) for l in range(D)
                     for bi in range(BPC) for g in range(2)]
            pend = None
            for j, (l, bi, g) in enumerate(units):
                if j == 0:
                    conv_dma(0, 2)
                if j == 1:
                    conv_dma(0, 3)
                    for piece in (lambda: conv_qk(0, 2, 0),
                                  lambda: conv_qk(0, 2, 1),
                                  lambda: conv_v(0, 2, 0),
                                  lambda: conv_v(0, 2, 1)):
                        THUNKS.append(piece)
                if j == 2:
                    for piece in (lambda: conv_qk(0, 3, 0),
                                  lambda: conv_qk(0, 3, 1),
                                  lambda: conv_v(0, 3, 0),
                                  lambda: conv_v(0, 3, 1)):
                        THUNKS.append(piece)
                if j % 8 == 2 and l + 1 < D:
                    wsb[l + 1] = load_weights(l + 1)
                enqueue_sched(j)
                st_new = {'u': (l, bi, g), 'eks': []}
                for k in range(4):
                    scores_k(l, bi, g, k, st_new['eks'])
                    if pend is not None:
                        attend_k(pend, k)
                    if THUNKS:
                        THUNKS.popleft()()
                if pend is not None:
                    attend_fin(pend)
                pend = st_new
            for j in sorted(sched.keys()):
                enqueue_sched(j)
            for k in range(4):
                attend_k(pend, k)
                if THUNKS:
                    THUNKS.popleft()()
            attend_fin(pend)
            while THUNKS:
                THUNKS.popleft()()

            # ---------- sigmoid + output ----------
            r128 = psm.tile([128, 16], FP32, tag='r128', bufs=1)
            nc.sync.dma_start(out=r128, in_=l4c)
            sg = psm.tile([128, 16], FP32, tag='sg', bufs=1)
            nc.scalar.activation(out=sg, in_=r128, func=AF.Exp, bias=0.0,
                                 scale=-1.0)
            nc.vector.tensor_scalar(out=sg, in0=sg, scalar1=1.0, scalar2=None,
                                    op0=ALU.add)
            rr = psm.tile([128, 16], FP32, tag='rr', bufs=1)
            nc.vector.reciprocal(out=rr, in_=sg)
            nc.sync.dma_start(out=out_d[:, :], in_=rr)

    # populate .instr bytes for extended-inst InstISA subclasses (the
    # custom-DVE reciprocal_approx_fast) — raw Bass skips this pass and
    # the NEFF compiler rejects empty .instr as "ISA wrong length"
    mybir.codegen_inst_isa_subclasses(nc)
    if split_waits:
        _split_excess_waits(nc, mybir, {'InstNoOp': 99})
    return nc


def _get_nc():
    if 'nc' not in _CACHE:
        _CACHE['nc'] = build_nc(debug=False)
    return _CACHE['nc']


def kernel(**inputs) -> np.ndarray:
    from concourse.bass_utils import run_bass_kernel_spmd

    nc = _get_nc()
    in_maps = _host_prep(inputs)
    res = run_bass_kernel_spmd(nc, in_maps, list(range(NCORES)))
    return np.concatenate([r['out'] for r in res.results], axis=0)



# revision 27
# speedup vs baseline: 1.0069x; 1.0069x over previous
"""Trainium2 Bass kernel for nn_DDSTSTransformer_9663676416718.

Data-parallel over batch B=32 across 8 cores (4 batches/core), params
replicated. Per-core activations live as [128, 512] tiles = 4 batches
stacked along partitions (32 channels each).

Structure (v3):
  - The scalar engine's exp over the full [512,512] score matrix per
    (batch, head) is the primary roof (~15-18 us per batch-layer); the
    in-order PE at HAM-throttled clocks is a close co-roof. ~3 of 16
    exp tiles per batch-layer are offloaded to the vector engine via a
    Schraudolph fast-exp (one mult+add tensor_scalar emitting bf16 bits
    through an int16 convert; the softmax ratio cancels the ~1.5%
    per-element error).
  - DDS convs are host-fused into 15-tap full convs (bf16) and run as
    im2col matmuls over a gather tile that the SWDGE conv-dma casts
    fp32->bf16 in flight.
  - Scores S = K^T K (bf16) run row-tiled (4 heads concurrent); AV and
    the ones-matmul Z run col-tiled (4 heads concurrent) into separate
    1-bank av/zz tiles; divide is reciprocal_approx_fast + tensor_mul.
  - PSUM (8 banks): score tiles rotate in a dedicated 2-deep ring
    (tag 's', 4 banks), av/zz in a 2-deep 1-bank ring (tag 'z'), and
    conv/tail/logits tiles in a 2-deep 1-bank ring (tag 'cv').
  - Emission interleaves per key-tile: [scores_k(u) | AV_k(u-1) | one
    background thunk], where tails and next-layer convs are split into
    small thunks popped one per k-position from a FIFO, so the exp
    stream is never parked behind a long PE stretch.
"""

import numpy as np

B, CIN, L = 32, 6, 512
TC, H, D, DM = 32, 8, 6, 4
EPS = 1e-5
NCORES = 8
BPC = B // NCORES  # batches per core
KT = 15            # fused dds conv taps
PAD = (KT - 1) // 2

# Conv weights (bf16 wconv blob, [128, 2048]): stored for im2col
# matmuls: partition = (tt, c) with tap t = 4*k + tt (padded to 16 taps,
# tap 15 zero), k in the free dim.
QKOFF = 0                  # [4k, 256] fused qk conv im2col lhsT
VOFF = QKOFF + 4 * 256     # [4k, 256] fused v conv im2col rhs
NWC = VOFF + 4 * 256
# fp32 wrest blob [128, NWR]
FF1OFF = 0                 # [128] ff1 lhsT, band-replicated
FF2OFF = FF1OFF + 128      # [32]  ff2 lhsT (full 128 partitions)
UOFF = FF2OFF + 32         # [2, 32] unify lhsT per ktile
GBOFF = UOFF + 64          # 4 cols: n1_g, n1_b, n2_g, n2_b (tiled x4)
NWR = GBOFF + 4
XPW = L + 16               # padded input width (7 left, 9 right zeros)

_CACHE = {}


def _split_excess_waits(nc, mybir, limits):
    """Walrus's TPB encodings accept a limited number of sync-wait
    commands per instruction (1 for Matmult/LDWEIGHTS on this build).
    Move excess waits onto freshly inserted same-engine NoOps directly
    before the instruction — identical engine-stream semantics, valid
    encoding."""
    for fn in nc.m.functions:
        for blk in fn.blocks:
            il = blk.instructions
            i = 0
            while i < len(il):
                inst = il[i]
                keep = limits.get(type(inst).__name__, 1)
                si = getattr(inst, 'sync_info', None)
                if si is not None and si.on_wait and len(si.on_wait) > keep:
                    waits = list(si.on_wait)
                    extra, rest = waits[:-keep], waits[-keep:]
                    nops = []
                    for w in extra:
                        n = mybir.InstNoOp(name=f'I-wsplit-{nc.next_id()}',
                                           ins=[], outs=[])
                        n.engine = inst.engine
                        n.sync_info = mybir.SyncInfo(on_wait=[w], on_update=[])
                        nops.append(n)
                    inst.sync_info = mybir.SyncInfo(
                        on_wait=rest, on_update=list(si.on_update))
                    il[i:i] = nops
                    i += len(nops)
                i += 1


def _fuse_dds(pw, dwa, dwb, gate):
    """Fold pointwise + gated depthwise pair into W[t, c, o]."""
    g = np.exp(gate - gate.max())
    g = g / g.sum()
    O = pw.shape[0]
    ka, kb = dwa.shape[1], dwb.shape[1]
    k = max(ka, kb)
    wc = np.zeros((O, k), np.float32)
    wc[:, (k - ka) // 2:(k - ka) // 2 + ka] += g[0] * dwa
    wc[:, (k - kb) // 2:(k - kb) // 2 + kb] += g[1] * dwb
    # W[t, c, o] = pw[o, c] * wc[o, t]
    return np.einsum('oc,ot->tco', pw, wc).astype(np.float32)


def _rep4(a):
    """Tile a [32, ...] band 4x along partitions -> [128, ...]."""
    return np.concatenate([a] * 4, axis=0)


def _host_prep(inputs):
    """Build per-core input maps (numpy only)."""
    import ml_dtypes
    BF = ml_dtypes.bfloat16
    x = np.asarray(inputs['x'], np.float32)

    wconv = np.zeros((D, 128, NWC), BF)
    wrest = np.zeros((D, 128, NWR), np.float32)
    for l in range(D):
        wqk = _fuse_dds(np.asarray(inputs['qk_pw'][l]), np.asarray(inputs['qk_dwa'][l]),
                        np.asarray(inputs['qk_dwb'][l]), np.asarray(inputs['qk_gate'][l]))
        wv = _fuse_dds(np.asarray(inputs['v_pw'][l]), np.asarray(inputs['v_dwa'][l]),
                       np.asarray(inputs['v_dwb'][l]), np.asarray(inputs['v_gate'][l]))
        # [t, c, o] -> partition (tt, c) = 32*tt + c, free (k, o) = 256*k + o
        for W, off in ((wqk, QKOFF), (wv, VOFF)):
            W16 = np.zeros((16, TC, 256), np.float32)
            W16[:KT] = W
            if l == 0:
                # compose the pointwise encoder into the layer-0 convs so
                # they read the raw input directly (encoder stays only on
                # the residual path, off the startup critical chain)
                Wc = np.einsum('tco,cd->tdo', W16,
                               np.asarray(inputs['enc_w'], np.float32))
                W16 = np.zeros((16, TC, 256), np.float32)
                W16[:, :CIN] = Wc
            wconv[l, :, off:off + 1024] = (
                W16.reshape(4, 4, TC, 256)        # [k, tt, c, o]
                   .transpose(1, 2, 0, 3)         # [tt, c, k, o]
                   .reshape(128, 1024).astype(BF))
        wrest[l, :, FF1OFF:FF1OFF + 128] = _rep4(np.asarray(inputs['ff_w1'][l]).T)   # [c, m]
        wrest[l, :, FF2OFF:FF2OFF + 32] = np.asarray(inputs['ff_w2'][l]).T           # [dm, o]
        # unify lhsT: u[hc_lo, kt*32+o] = unify_w[o, kt*128+hc_lo]
        ut = np.asarray(inputs['unify_w'][l]).T.reshape(2, 128, TC).transpose(1, 0, 2)
        wrest[l, :, UOFF:UOFF + 64] = ut.reshape(128, 64)
        wrest[l, :, GBOFF + 0] = _rep4(np.asarray(inputs['n1_g'][l]))
        wrest[l, :, GBOFF + 1] = _rep4(np.asarray(inputs['n1_b'][l]))
        wrest[l, :, GBOFF + 2] = _rep4(np.asarray(inputs['n2_g'][l]))
        wrest[l, :, GBOFF + 3] = _rep4(np.asarray(inputs['n2_b'][l]))

    encw = np.zeros((128, TC), np.float32)
    for i in range(4):
        encw[32 * i:32 * i + CIN, :] = np.asarray(inputs['enc_w']).T  # [c, o]

    wlog = _fuse_dds(np.asarray(inputs['log_pw']), np.asarray(inputs['log_dwa']),
                     np.asarray(inputs['log_dwb']), np.asarray(inputs['log_gate']))  # [3, 32, 1]
    wtail = _rep4(wlog[:, :, 0].T)  # [c, t] -> [128, 3]

    in_maps = []
    for core in range(NCORES):
        xin4 = np.zeros((128, L), np.float32)
        for i in range(BPC):
            xin4[32 * i:32 * i + CIN, :] = x[BPC * core + i]
        in_maps.append({
            'xin4': xin4,
            'encw': encw,
            'wconv': wconv,
            'wrest': wrest,
            'wtail': wtail,
        })
    return in_maps


def build_nc(debug=False, split_waits=True):
    import concourse.bass as bass
    import concourse.mybir as mybir
    import concourse.tile as tile

    FP32 = mybir.dt.float32
    BF16 = mybir.dt.bfloat16
    AF = mybir.ActivationFunctionType
    ALU = mybir.AluOpType
    INV_SQRT_C = float(TC) ** -0.5
    F32R = mybir.dt.float32r
    I16 = mybir.dt.int16
    SCH_A = (128.0 / np.log(2.0)) * INV_SQRT_C
    SCH_B = 127.0 * 128.0 - 8.5

    nc = bass.Bass()
    xin4_d = nc.declare_dram_parameter('xin4', [128, L], FP32, isOutput=False)
    encw_d = nc.declare_dram_parameter('encw', [128, TC], FP32, isOutput=False)
    wconv_d = nc.declare_dram_parameter('wconv', [D, 128, NWC], BF16, isOutput=False)
    wrest_d = nc.declare_dram_parameter('wrest', [D, 128, NWR], FP32, isOutput=False)
    wtail_d = nc.declare_dram_parameter('wtail', [128, 3], FP32, isOutput=False)
    out_d = nc.declare_dram_parameter('out', [BPC, L], FP32, isOutput=True)
    dbg = {}
    if debug:
        for name, shape in [('dbg_enc', [128, L]), ('dbg_qk', [128, L]),
                            ('dbg_vt', [128, 256]), ('dbg_exps', [128, L]),
                            ('dbg_z', [128, L]),
                            ('dbg_attn', [128, L]), ('dbg_l0', [128, L])]:
            dbg[name] = nc.declare_dram_parameter(name, shape, FP32, isOutput=True)

    with tile.TileContext(nc) as tc:
        with (
            tc.tile_pool(name='pconst', bufs=1) as pconst,
            tc.tile_pool(name='pw', bufs=3) as pw,
            tc.tile_pool(name='pqk', bufs=12) as pqk,
            tc.tile_pool(name='pvt', bufs=20) as pvt,
            tc.tile_pool(name='pek', bufs=8) as pek,
            tc.tile_pool(name='pat', bufs=5) as pat,
            tc.tile_pool(name='prz', bufs=2) as prz,
            tc.tile_pool(name='pa', bufs=4) as pa,
            tc.tile_pool(name='ppad', bufs=2) as ppad,
            tc.tile_pool(name='pxc', bufs=3) as pxc,
            tc.tile_pool(name='psm', bufs=4) as psm,
            tc.tile_pool(name='pps', bufs=2, space='PSUM') as pps,
            tc.tile_pool(name='pz', bufs=2, space='PSUM') as pz,
            tc.tile_pool(name='pcv', bufs=2, space='PSUM') as pcv,
        ):

            def mm(out, lhsT, rhs, **kw):
                # float32r streams 1 col/cycle at N>=256 (fp32 is 4x slower)
                nc.tensor.matmul(out=out, lhsT=lhsT.bitcast(F32R),
                                 rhs=rhs.bitcast(F32R), **kw)

            eps_t = pconst.tile([128, 1], FP32, tag='eps')
            nc.vector.memset(eps_t, EPS)
            zeros16 = pconst.tile([128, 16], FP32, tag='zeros16')
            nc.vector.memset(zeros16, 0.0)
            ones32 = pconst.tile([128, 32], BF16, tag='ones32')
            nc.vector.memset(ones32, 1.0)
            xin4 = pconst.tile([128, L], FP32, tag='xin4')
            encw = pconst.tile([128, TC], FP32, tag='encw')
            wtail = pconst.tile([128, 3], FP32, tag='wtail')
            nc.sync.dma_start(out=xin4.bitcast(F32R), in_=xin4_d[:, :].bitcast(F32R))

            # preload the natural_log_exp ACT table set during startup DMAs
            warm = psm.tile([128, 1], FP32, tag='warm', bufs=1)
            nc.scalar.activation(out=warm, in_=eps_t, func=AF.Exp, bias=0.0,
                                 scale=1.0)

            def load_weights(l):
                wc = pw.tile([128, NWC], BF16, tag='wcsb')
                for a, b in ((QKOFF, VOFF), (VOFF, NWC)):
                    nc.sync.dma_start(out=wc[:, a:b], in_=wconv_d[l, :, a:b])
                wr = pw.tile([128, NWR], FP32, tag='wrsb')
                nc.sync.dma_start(out=wr.bitcast(F32R),
                                  in_=wrest_d[l, :, :].bitcast(F32R))
                return (wc, wr)

            wsb = {0: load_weights(0)}
            # encw/wtail after layer-0 weights: nothing needs them early
            nc.sync.dma_start(out=encw.bitcast(F32R), in_=encw_d[:, :].bitcast(F32R))
            nc.sync.dma_start(out=wtail.bitcast(F32R), in_=wtail_d[:, :].bitcast(F32R))

            # ---------- encoder ----------
            enct = pcv.tile([128, L], FP32, tag='cv', name='enct')
            encp = enct
            for i in range(4):
                nc.tensor.matmul(
                    out=encp[32 * i:32 * i + 32, :],
                    lhsT=encw[32 * i:32 * i + 32, :],
                    rhs=xin4[32 * i:32 * i + 32, :],
                    start=True, stop=True,
                    tile_position=(32 * i, 32 * i))
            enc4 = pa.tile([128, L], FP32, tag='out4', bufs=4, name='enc4')
            nc.vector.tensor_copy(out=enc4, in_=encp)
            if debug:
                nc.sync.dma_start(out=dbg['dbg_enc'][:, :], in_=enc4)

            # pipeline state
            OUT4 = {0: enc4}   # OUT4[l] = input residual stream for layer l
            MID = {}           # MID[l] = post-norm1 stream of layer l
            Q, VT, XP, AT = {}, {}, {}, {}

            def build_xpad_band(l, bi):
                if l not in XP:
                    XP[l] = ppad.tile([128, XPW], FP32, tag='xpad',
                                      name=f'xpad{l}')
                xp = XP[l]
                src = xin4 if l == 0 else OUT4[l]
                p0, p1 = 32 * bi, 32 * bi + 32
                nc.vector.tensor_copy(out=xp[p0:p1, 0:PAD].bitcast(F32R),
                                      in_=zeros16[0:32, 0:PAD])
                nc.vector.tensor_copy(out=xp[p0:p1, PAD + L:].bitcast(F32R),
                                      in_=zeros16[0:32, 0:XPW - PAD - L])
                nc.vector.tensor_copy(out=xp[p0:p1, PAD:PAD + L].bitcast(F32R),
                                      in_=src[p0:p1, :])
                return xp

            XC = {}

            def conv_dma(l, bi):
                """im2col gather for (layer l, batch bi) — prefetch stage.
                gpsimd (SWDGE) so the gather casts fp32 -> bf16 in flight."""
                xp = build_xpad_band(l, bi)
                xc = pxc.tile([128, 4, L], BF16, tag='xc', name=f'xc{l}_{bi}')
                for tt in range(4):
                    sl = xp[32 * bi:32 * bi + 32, tt:tt + L]
                    src_ap = bass.AP(
                        tensor=sl.tensor, offset=sl.offset,
                        ap=[sl.ap[0], [4, 4], [1, L]])
                    nc.gpsimd.dma_start(out=xc[32 * tt:32 * tt + 32, :, :],
                                        in_=src_ap)
                XC[(l, bi)] = xc

            def conv_qk(l, bi, j):
                """qk conv half j for (l, bi): 4 matmuls + bf16 drain."""
                w, _ = wsb[l]
                xc = XC[(l, bi)]
                qp = pcv.tile([128, L], FP32, tag='cv', name=f'qp{l}_{bi}_{j}')
                for k in range(4):
                    nc.tensor.matmul(
                       out=qp,
                       lhsT=w[:, QKOFF + k * 256 + j * 128:
                              QKOFF + k * 256 + j * 128 + 128],
                       rhs=xc[:, k, :],
                       start=(k == 0), stop=(k == 3))
                q = pqk.tile([128, L], BF16, tag='qk', name=f'q{l}_{bi}_{j}')
                nc.vector.tensor_copy(out=q, in_=qp)
                Q.setdefault((l, bi), []).append(q)

            def conv_v(l, bi, half):
                """v conv half (2 l-tiles) for (l, bi), transposed out."""
                w, _ = wsb[l]
                xc = XC[(l, bi)]
                vp = pcv.tile([128, 2, 256], FP32, tag='cv',
                              name=f'vp{l}_{bi}_{half}')
                for i in range(2):
                    lt = 2 * half + i
                    for k in range(4):
                        nc.tensor.matmul(
                           out=vp[:, i, :],
                           lhsT=xc[:, k, 128 * lt:128 * lt + 128],
                           rhs=w[:, VOFF + k * 256:VOFF + (k + 1) * 256],
                           start=(k == 0), stop=(k == 3))
                for i in range(2):
                    # bf16: walrus rejects fp32r col-tiled matmuls, and the
                    # AV/Z stage col-tiles; cast is free in this PSUM drain
                    v = pvt.tile([128, 256], BF16, tag='vt',
                                 name=f'v{l}_{bi}_{2 * half + i}')
                    nc.vector.tensor_copy(out=v, in_=vp[:, i, :])
                    VT.setdefault((l, bi), []).append(v)
                if half == 1:
                    XC.pop((l, bi))

            def scores_k(l, bi, g, k, eks):
                """Score matmuls + exp for head group g, key-tile k."""
                qk = Q[(l, bi)][g]
                psA = pps.tile([128, 2, L], FP32, tag='s',
                               name=f'sA{l}_{bi}_{g}_{k}')
                psB = pps.tile([128, 2, L], FP32, tag='s',
                               name=f'sB{l}_{bi}_{g}_{k}')
                for hh in range(4):
                    ps, half = (psA, hh) if hh < 2 else (psB, hh - 2)
                    nc.tensor.matmul(
                       out=ps[:, half, :],
                       lhsT=qk[32 * hh:32 * hh + 32, 128 * k:128 * k + 128],
                       rhs=qk[32 * hh:32 * hh + 32, :],
                       start=True, stop=True,
                       tile_position=(32 * hh, 0))
                ekk = pek.tile([128, 4, L], BF16, tag='ek',
                               name=f'ek{l}_{bi}_{g}_{k}')
                nc.scalar.activation(out=ekk[:, 0:2, :],
                                     in_=psA[:, :, :], func=AF.Exp,
                                     bias=0.0, scale=INV_SQRT_C)
                if k == 3 or (k == 1 and g == 0):
                    # Schraudolph exp on DVE, directly in bf16 bit space:
                    # bf16_bits(e^x) ~ round(128/ln2 * x + (127*128 - 8.5)).
                    # The softmax ratio cancels the ~1.5% element error.
                    nc.vector.tensor_scalar(
                        out=ekk[:, 2:4, :].bitcast(I16),
                        in0=psB[:, :, :],
                        scalar1=SCH_A, scalar2=SCH_B,
                        op0=ALU.mult, op1=ALU.add)
                else:
                    nc.scalar.activation(out=ekk[:, 2:4, :],
                                         in_=psB[:, :, :], func=AF.Exp,
                                         bias=0.0, scale=INV_SQRT_C)
                eks.append(ekk)

            def attend_k(st, k):
                """Col-tiled AV + replicated Z for key-tile k of unit st."""
                l, bi, g, eks = st['u'] + (st['eks'],)
                vt = VT[(l, bi)]
                if k == 0:
                    st['av'] = pz.tile([128, L], FP32, tag='z',
                                       name=f'av{l}_{bi}_{g}')
                    st['zz'] = pz.tile([128, L], FP32, tag='z',
                                       name=f'zz{l}_{bi}_{g}')
                av, zz = st['av'], st['zz']
                for h in range(4):
                    nc.tensor.matmul(
                        out=av[32 * h:32 * h + 32, :],
                        lhsT=vt[k][:, 32 * (4 * g + h):32 * (4 * g + h) + 32],
                        rhs=eks[k][:, h, :],
                        start=(k == 0), stop=(k == 3),
                        tile_position=(0, 32 * h))
                for h in range(4):
                    nc.tensor.matmul(
                        out=zz[32 * h:32 * h + 32, :],
                        lhsT=ones32,
                        rhs=eks[k][:, h, :],
                        start=(k == 0), stop=(k == 3),
                        tile_position=(0, 32 * h))

            def attend_fin(st):
                """Softmax divide: at = av * (1/zz)."""
                l, bi, g = st['u']
                rzb = prz.tile([128, L], FP32, tag='rz')
                nc.vector.reciprocal_approx_fast(out=rzb, in_=st['zz'])
                at = pat.tile([128, L], FP32, tag='at', name=f'at{l}_{bi}_{g}')
                nc.vector.tensor_mul(out=at.bitcast(F32R), in0=st['av'],
                                     in1=rzb)
                AT[(bi, g)] = at

            def norm_band(resb, dst, bi, w, goff, boff):
                """InstanceNorm on one 32-partition band: dst_band =
                resb*se + (beta - mu*se), se = gamma/sqrt(var+eps)."""
                p0, p1 = 32 * bi, 32 * bi + 32
                stats = psm.tile([128, 6], FP32, tag='stats', bufs=4)
                mv = psm.tile([128, 2], FP32, tag='mv', bufs=4)
                nc.vector.bn_stats(out=stats[p0:p1, :], in_=resb[p0:p1, :])
                nc.vector.bn_aggr(out=mv[p0:p1, :], in_=stats[p0:p1, :])
                lnv = psm.tile([128, 1], FP32, tag='lnv', bufs=4)
                rstd = psm.tile([128, 1], FP32, tag='rstd', bufs=4)
                nc.scalar.activation(out=lnv[p0:p1, :], in_=mv[p0:p1, 1:2],
                                     func=AF.Ln, bias=eps_t[p0:p1, :], scale=1.0)
                nc.scalar.activation(out=rstd[p0:p1, :], in_=lnv[p0:p1, :],
                                     func=AF.Exp, bias=0.0, scale=-0.5)
                se = psm.tile([128, 1], FP32, tag='se', bufs=4)
                tmp = psm.tile([128, 1], FP32, tag='tmp', bufs=4)
                bv = psm.tile([128, 1], FP32, tag='bv', bufs=4)
                nc.vector.tensor_mul(out=se[p0:p1, :], in0=rstd[p0:p1, :],
                                     in1=w[p0:p1, goff:goff + 1])
                nc.vector.tensor_mul(out=tmp[p0:p1, :], in0=mv[p0:p1, 0:1],
                                     in1=se[p0:p1, :])
                nc.vector.tensor_sub(out=bv[p0:p1, :], in0=w[p0:p1, boff:boff + 1],
                                     in1=tmp[p0:p1, :])
                nc.vector.tensor_scalar(out=dst[p0:p1, :].bitcast(F32R),
                                        in0=resb[p0:p1, :],
                                        scalar1=se[p0:p1, :],
                                        scalar2=bv[p0:p1, :],
                                        op0=ALU.mult, op1=ALU.add)

            def tail_a(l, bi):
                """unify + residual + norm1 for one batch band."""
                _, w = wsb[l]
                ub = pcv.tile([128, L], FP32, tag='cv', name=f'ub{l}_{bi}')
                for kt in range(2):
                    mm(out=ub[0:32, :],
                       lhsT=w[:, UOFF + kt * 32:UOFF + (kt + 1) * 32],
                       rhs=AT[(bi, kt)],
                       start=(kt == 0), stop=(kt == 1))
                if l not in MID:
                    MID[l] = pa.tile([128, L], FP32, tag='out4', bufs=4,
                                     name=f'mid{l}')
                resb = pa.tile([128, L], FP32, tag='res', bufs=3,
                               name=f'r1_{l}_{bi}')
                p0, p1 = 32 * bi, 32 * bi + 32
                nc.vector.tensor_add(out=resb[p0:p1, :], in0=ub[0:32, :],
                                     in1=OUT4[l][p0:p1, :])
                norm_band(resb, MID[l], bi, w, GBOFF + 0, GBOFF + 1)

            FFS = {}

            def tail_b1(l, bi):
                """ff expand + relu for one batch band."""
                _, w = wsb[l]
                p0, p1 = 32 * bi, 32 * bi + 32
                ffp = pcv.tile([128, L], FP32, tag='cv', name=f'ffp{l}_{bi}')
                mm(out=ffp,
                   lhsT=w[p0:p1, FF1OFF:FF1OFF + 128],
                   rhs=MID[l][p0:p1, :],
                   start=True, stop=True,
                   tile_position=(32 * bi, 0))
                ffs = pa.tile([128, L], FP32, tag='ffs', bufs=2,
                              name=f'ffs{l}_{bi}')
                nc.vector.tensor_scalar_max(out=ffs.bitcast(F32R), in0=ffp,
                                            scalar1=0.0)
                FFS[(l, bi)] = ffs


            def tail_b2(l, bi):
                """ff contract + residual + norm2 for one batch band."""
                _, w = wsb[l]
                p0, p1 = 32 * bi, 32 * bi + 32
                f2 = pcv.tile([128, L], FP32, tag='cv', name=f'f2_{l}_{bi}')
                mm(out=f2[0:32, :],
                   lhsT=w[:, FF2OFF:FF2OFF + 32],
                   rhs=FFS.pop((l, bi)),
                   start=True, stop=True)
                if (l + 1) not in OUT4:
                    OUT4[l + 1] = pa.tile([128, L], FP32, tag='out4', bufs=4,
                                          name=f'out{l + 1}')
                resb = pa.tile([128, L], FP32, tag='res', bufs=3,
                               name=f'r2_{l}_{bi}')
                nc.vector.tensor_add(out=resb[p0:p1, :], in0=f2[0:32, :],
                                     in1=MID[l][p0:p1, :])
                norm_band(resb, OUT4[l + 1], bi, w, GBOFF + 2, GBOFF + 3)

            l4c = pa.tile([1, BPC * L], FP32, tag='l4', bufs=1)

            def logits(bi):
                """to_logits DDS conv (3 taps) for one batch."""
                xp = build_xpad_band(D, bi)
                pst = pcv.tile([128, L], FP32, tag='cv', name=f'tail{bi}')
                for t in range(3):
                    mm(out=pst[0:1, :],
                       lhsT=wtail[32 * bi:32 * bi + 32, t:t + 1],
                       rhs=xp[32 * bi:32 * bi + 32, PAD - 1 + t:PAD - 1 + t + L],
                       start=(t == 0), stop=(t == 2),
                       tile_position=(32 * bi, 0))
                nc.vector.tensor_copy(out=l4c[0:1, bi * L:(bi + 1) * L],
                                      in_=pst[0:1, :])

            # ---------- pipelined emission (global flat schedule) ----------
            # per-k interleave: the PE stream alternates [scores_k(u) |
            # AV_k(u-1) | one background thunk], so the depth-2 score ring
            # never parks the exp stream and background work (tails, convs)
            # fills the PE while ACT drains each score tile. Convs are
            # split into 4 thunks, tails into 3; im2col DMAs prefetch one
            # slot ahead of the matmul thunks.
            from collections import deque
            THUNKS = deque()

            sched = {}
            for l in range(D):
                for b in range(BPC):
                    o = 8 * l + 2 * b
                    if l < D - 1:
                        sched.setdefault(o + 3, []).append(('ta', l, b))
                        sched.setdefault(o + 4, []).append(('tb', l, b))
                        sched.setdefault(o + 5, []).append(('nsd', l + 1, b))
                        sched.setdefault(o + 6, []).append(('nsm', l + 1, b))
                    else:
                        # last layer: pull tails/logits as early as the
                        # data deps allow, so only batch 3's chain trails
                        # the final exp
                        sched.setdefault(o + 2, []).append(('ta', l, b))
                        sched.setdefault(o + 3, []).append(('tb', l, b))
                        sched.setdefault(o + 3, []).append(('nsd', D, b))
                        sched.setdefault(o + 3, []).append(('nsm', D, b))

            def enqueue_sched(j):
                for kind, a0, a1 in sched.pop(j, []):
                    if kind == 'ta':
                        THUNKS.append(lambda a0=a0, a1=a1: tail_a(a0, a1))
                    elif kind == 'tb':
                        THUNKS.append(lambda a0=a0, a1=a1: tail_b1(a0, a1))
                        THUNKS.append(lambda a0=a0, a1=a1: tail_b2(a0, a1))
                    elif kind == 'nsd':
                        if a0 < D:
                            THUNKS.append(
                                lambda a0=a0, a1=a1: conv_dma(a0, a1))
                        else:
                            THUNKS.append(
                                lambda a1=a1: build_xpad_band(D, a1))
                    elif kind == 'cqk':
                        for jj in (0, 1):
                            THUNKS.append(
                                lambda a0=a0, a1=a1, jj=jj:
                                    conv_qk(a0, a1, jj))
                    elif kind == 'cv':
                        for jj in (0, 1):
                            THUNKS.append(
                                lambda a0=a0, a1=a1, jj=jj:
                                    conv_v(a0, a1, jj))
                    else:
                        if a0 < D:
                            for jj, fn in ((0, conv_qk), (1, conv_qk),
                                           (0, conv_v), (1, conv_v)):
                                THUNKS.append(
                                    lambda a0=a0, a1=a1, jj=jj, fn=fn:
                                        fn(a0, a1, jj))
                        else:
                            THUNKS.append(lambda a1=a1: logits(a1))

            def conv_all(l, bi):
                conv_qk(l, bi, 0)
                conv_qk(l, bi, 1)
                conv_v(l, bi, 0)
                conv_v(l, bi, 1)

            # layer-0 convs read xin4 directly (encoder composed on host)
            conv_dma(0, 0)
            conv_all(0, 0)
            conv_dma(0, 1)
            conv_all(0, 1)
            units = [(l, bi, g) for l in range(D)
                     for bi in range(BPC) for g in range(2)]
            pend = None
            for j, (l, bi, g) in enumerate(units):
                if j == 0:
                    conv_dma(0, 2)
                if j == 1:
                    conv_dma(0, 3)
                    for piece in (lambda: conv_qk(0, 2, 0),
                                  lambda: conv_qk(0, 2, 1),
                                  lambda: conv_v(0, 2, 0),
                                  lambda: conv_v(0, 2, 1)):
                        THUNKS.append(piece)
                if j == 2:
                    for piece in (lambda: conv_qk(0, 3, 0),
                                  lambda: conv_qk(0, 3, 1),
                                  lambda: conv_v(0, 3, 0),
                                  lambda: conv_v(0, 3, 1)):
                        THUNKS.append(piece)
                if j % 8 == 2 and l + 1 < D:
                    wsb[l + 1] = load_weights(l + 1)
                enqueue_sched(j)
                st_new = {'u': (l, bi, g), 'eks': []}
                for k in range(4):
                    scores_k(l, bi, g, k, st_new['eks'])
                    if pend is not None:
                        attend_k(pend, k)
                    if THUNKS:
                        THUNKS.popleft()()
                if pend is not None:
                    attend_fin(pend)
                pend = st_new
            for j in sorted(sched.keys()):
                enqueue_sched(j)
            for k in range(4):
                attend_k(pend, k)
                if THUNKS:
                    THUNKS.popleft()()
            attend_fin(pend)
            while THUNKS:
                THUNKS.popleft()()

            # ---------- sigmoid + output ----------
            r128 = psm.tile([128, 16], FP32, tag='r128', bufs=1)
            nc.sync.dma_start(out=r128, in_=l4c)
            sg = psm.tile([128, 16], FP32, tag='sg', bufs=1)
            nc.scalar.activation(out=sg, in_=r128, func=AF.Exp, bias=0.0,
                                 scale=-1.0)
            nc.vector.tensor_scalar(out=sg, in0=sg, scalar1=1.0, scalar2=None,
                                    op0=ALU.add)
            rr = psm.tile([128, 16], FP32, tag='rr', bufs=1)
            nc.vector.reciprocal(out=rr, in_=sg)
            nc.sync.dma_start(out=out_d[:, :], in_=rr)

    # populate .instr bytes for extended-inst InstISA subclasses (the
    # custom-DVE reciprocal_approx_fast) — raw Bass skips this pass and
    # the NEFF compiler rejects empty .instr as "ISA wrong length"
    mybir.codegen_inst_isa_subclasses(nc)
    if split_waits:
        _split_excess_waits(nc, mybir, {'InstNoOp': 99})
    return nc


def _get_nc():
    if 'nc' not in _CACHE:
        _CACHE['nc'] = build_nc(debug=False)
    return _CACHE['nc']


def kernel(**inputs) -> np.ndarray:
    from concourse.bass_utils import run_bass_kernel_spmd

    nc = _get_nc()
    in_maps = _host_prep(inputs)
    res = run_bass_kernel_spmd(nc, in_maps, list(range(NCORES)))
    return np.concatenate([r['out'] for r in res.results], axis=0)

